# revision 10
# baseline (speedup 1.0000x reference)
"""TRN2 Bass kernel for nn_DkmCompGraph (vq_codebook).

reference:
    d2[n,k] = ||x_n||^2 + ||c_k||^2 - 2 x_n.c_k          (N=32768, K=1024, D=512)
    idx = argmin_k d2                                     -> (N,) int32
    centroids = cluster_rep[idx]                          -> (N, D) f32
    loss = sum(softmax(-d2, axis=1) * d2) / K             -> scalar f32

Strategy (8 cores, data-parallel over N; 4096 rows/core):
  - PE computes r = -d2/2 = x.c - ||x||^2/2 - ||c||^2/2 in PSUM via fp16
    hi/lo 3-term split matmuls (xh.ch + xh.cl + xl.ch, fp32-class accuracy)
    plus a K=3 bias matmul folding -||x||^2/2 (rank-1) and -||c||^2/2 rows.
  - argmin d2 = argmax r via DVE max (top-8) + max_index (first-occurrence
    ties, matching jnp.argmin).
  - softmax: e = exp(2r - 2 rmax) on ACT (scale=2, per-partition bias),
    accum_out gives s = sum_k e for free.
  - t = sum_k e * d2 = sum_k -2 e r via one fused tensor_tensor_reduce.
  - loss_row = t/s accumulated per partition; host sums 8x128 partials / K.
  - centroids: gpsimd indirect DMA row-gather from cluster_rep in DRAM.

Host side only reshards/transposes/splits inputs and concatenates outputs.
"""

import sys

if "/opt/trn_rl_repo" not in sys.path:
    sys.path.insert(0, "/opt/trn_rl_repo")

import numpy as np

N, D, K = 32768, 512, 1024
NCORES = 8
NSHARD = N // NCORES        # 4096
PT = 128                    # points per tile
NT = NSHARD // PT           # 32 tiles per core
DC = D // 128               # 4 contraction chunks
GATHER_GROUP = 4            # tiles per indirect-gather batch

# matmul mode: "f16x3" (safe) or "f32r" (fast, ~13-bit mantissa)
MM_MODE = "f16x3"

_cached = {}


def _build_bass():
    from concourse import bacc
    import concourse.mybir as mybir
    import concourse.tile as tile
    from concourse.bass import IndirectOffsetOnAxis

    f16 = mybir.dt.float16
    f32 = mybir.dt.float32
    bf16 = mybir.dt.bfloat16
    f32r = mybir.dt.float32r
    AT = mybir.ActivationFunctionType
    OP = mybir.AluOpType

    nc = bacc.Bacc()

    if MM_MODE == "f16x3":
        d_xh = nc.dram_tensor("xh", [D, NSHARD], f16, kind="ExternalInput")
        d_xl = nc.dram_tensor("xl", [D, NSHARD], f16, kind="ExternalInput")
        d_ch = nc.dram_tensor("ch", [D, K], f16, kind="ExternalInput")
        d_cl = nc.dram_tensor("cl", [D, K], f16, kind="ExternalInput")
    else:
        d_xr = nc.dram_tensor("xr", [D, NSHARD], f32r, kind="ExternalInput")
        d_cr = nc.dram_tensor("cr", [D, K], f32r, kind="ExternalInput")
    d_ct = nc.dram_tensor("ct", [D, K], f32, kind="ExternalInput")   # cT fp32 (for c2)
    d_c = nc.dram_tensor("c", [K, D], f32, kind="ExternalInput")     # gather source

    d_cent = nc.dram_tensor("cent", [NSHARD, D], f32, kind="ExternalOutput")
    d_idx = nc.dram_tensor("idx", [PT, NT], mybir.dt.int32, kind="ExternalOutput")
    d_loss = nc.dram_tensor("loss_col", [PT, 1], f32, kind="ExternalOutput")

    with tile.TileContext(nc) as tc:
        with (
            tc.tile_pool(name="big", bufs=1) as big,        # persistent inputs
            tc.tile_pool(name="small", bufs=1) as small,    # persistent small
            tc.tile_pool(name="prep", bufs=2) as prep,      # transient prep
            tc.tile_pool(name="work", bufs=3) as work,      # per-tile work
            tc.tile_pool(name="cent", bufs=2) as centp,     # gather staging
            tc.tile_pool(name="ps", bufs=2, space="PSUM") as ps,
            tc.tile_pool(name="ps1", bufs=2, space="PSUM") as ps1,
            tc.tile_pool(name="psc", bufs=1, space="PSUM") as psc,
        ):
            # ---- persistent input loads (chunk-major [128, DC*n]) ----
            def load_T(dram, n, dt, tag):
                t = big.tile([128, DC * n], dt, tag=tag)
                nc.sync.dma_start(
                    t.rearrange("p (dc n) -> p dc n", dc=DC),
                    dram.rearrange("(dc p) n -> p dc n", p=128),
                )
                return t

            if MM_MODE == "f16x3":
                xh = load_T(d_xh, NSHARD, f16, "xh")
                xl = load_T(d_xl, NSHARD, f16, "xl")
                ch = load_T(d_ch, K, f16, "ch")
                cl = load_T(d_cl, K, f16, "cl")
            else:
                xr = load_T(d_xr, NSHARD, f32r, "xr")
                cr = load_T(d_cr, K, f32r, "cr")

            def chunk(t, dc, n, sl=slice(None)):
                return t[:, dc * n:(dc + 1) * n][:, sl]

            # ---- prep: c2row = sum_d c^2 (fp32 PE ones-matmul), bias rows ----
            ones_f32 = small.tile([128, 1], f32, tag="ones32")
            nc.vector.memset(ones_f32, 1.0)
            ones_f16 = small.tile([128, 1], f16, tag="ones16")
            nc.vector.memset(ones_f16, 1.0)

            c2_psum = psc.tile([1, K], f32)
            for dc in range(DC):
                ctc = prep.tile([128, K], f32, tag="ctc")
                nc.sync.dma_start(ctc, d_ct[dc * 128:(dc + 1) * 128, :])
                csq = prep.tile([128, K], f32, tag="csq")
                nc.scalar.activation(csq, ctc, AT.Square)
                for kh in range(2):
                    nc.tensor.matmul(
                        c2_psum[:, kh * 512:(kh + 1) * 512],
                        lhsT=ones_f32,
                        rhs=csq[:, kh * 512:(kh + 1) * 512],
                        start=(dc == 0),
                        stop=(dc == DC - 1),
                    )

            # bias_rhs [3, K] f16: rows = [ones, c2hi, c2lo] where c2hi+c2lo ~ -c2/2
            # (engine writes must start at partition 0/32/64/96, so compute the
            #  c2 rows in partition-0 tiles and DMA them into rows 1 and 2)
            bias_rhs = small.tile([3, K], f16, tag="brhs")
            nc.vector.memset(bias_rhs, 1.0)
            c2hi = small.tile([1, K], f16, tag="c2hi")
            nc.scalar.activation(c2hi, c2_psum, AT.Copy, scale=-0.5)
            c2lo_f32 = small.tile([1, K], f32, tag="c2lo")
            # c2lo = (-c2/2) - f16(-c2/2):  (scalar_tensor_tensor: (in0*-0.5) - hi)
            nc.vector.scalar_tensor_tensor(
                out=c2lo_f32,
                in0=c2_psum,
                scalar=-0.5,
                in1=c2hi,
                op0=OP.mult,
                op1=OP.subtract,
            )
            c2lo = small.tile([1, K], f16, tag="c2lo16")
            nc.vector.tensor_copy(c2lo, c2lo_f32)
            nc.sync.dma_start(bias_rhs[1:2, :], c2hi)
            nc.sync.dma_start(bias_rhs[2:3, :], c2lo)

            # bias_lhsT [3, PT] f16: rows = [negx2half (per tile), ones, ones]
            bias_lhsT = small.tile([3, PT], f16, tag="blhs")
            nc.vector.memset(bias_lhsT, 1.0)

            # in_max8 scaffold for max_index
            # (filled per-tile: col0 = rmax; cols 1..7 = -inf, never match)
            # built directly by nc.vector.max instead (true top-8).

            # per-row accumulators
            s_all = small.tile([PT, NT], f32, tag="s_all")
            t_all = small.tile([PT, NT], f32, tag="t_all")
            idx_g = small.tile([PT, NT], mybir.dt.int32, tag="idx_g")

            # ---- main loop ----
            for i in range(NT):
                psl = slice(i * PT, (i + 1) * PT)

                # x2 = sum_d xh^2 (f16): one ACT square + 4 accumulating ones-mms
                if MM_MODE == "f16x3":
                    sq_src = xh
                    sq_dt = f16
                else:
                    sq_src = xr
                    sq_dt = f16
                sq = work.tile([128, DC * PT], sq_dt, tag="sq")
                nc.scalar.activation(
                    sq.rearrange("p (dc n) -> p dc n", dc=DC),
                    sq_src.rearrange("p (dc n) -> p dc n", dc=DC)[:, :, psl],
                    AT.Square,
                )
                x2_psum = ps1.tile([1, PT], f32)
                for dc in range(DC):
                    nc.tensor.matmul(
                        x2_psum,
                        lhsT=ones_f16,
                        rhs=chunk(sq, dc, PT),
                        start=(dc == 0),
                        stop=(dc == DC - 1),
                    )
                # bias_lhsT row 0 = -x2/2 (f16)
                nc.scalar.activation(bias_lhsT[0:1, :], x2_psum, AT.Copy, scale=-0.5)

                # main matmuls -> r = -d2/2 in PSUM [128, K] f32
                r_psum = ps.tile([128, K], f32)
                for kh in range(2):
                    ksl = slice(kh * 512, (kh + 1) * 512)
                    if MM_MODE == "f16x3":
                        pairs = [
                            (a, b)
                            for dc in range(DC)
                            for (a, b) in (
                                (chunk(xh, dc, NSHARD, psl), chunk(ch, dc, K, ksl)),
                                (chunk(xh, dc, NSHARD, psl), chunk(cl, dc, K, ksl)),
                                (chunk(xl, dc, NSHARD, psl), chunk(ch, dc, K, ksl)),
                            )
                        ]
                    else:
                        pairs = [
                            (chunk(xr, dc, NSHARD, psl), chunk(cr, dc, K, ksl))
                            for dc in range(DC)
                        ]
                    for j, (a, b) in enumerate(pairs):
                        nc.tensor.matmul(
                            r_psum[:, ksl], lhsT=a, rhs=b,
                            start=(j == 0), stop=False,
                        )
                    nc.tensor.matmul(
                        r_psum[:, ksl],
                        lhsT=bias_lhsT,
                        rhs=bias_rhs[:, ksl],
                        start=False,
                        stop=True,
                    )

                # top-8 + argmax (first occurrence on ties = jnp.argmin order)
                max8 = work.tile([PT, 8], f32, tag="max8")
                nc.vector.max(out=max8, in_=r_psum)
                idx8 = work.tile([PT, 8], mybir.dt.uint32, tag="idx8")
                nc.vector.max_index(out=idx8, in_max=max8, in_values=r_psum)
                nc.vector.tensor_copy(
                    idx_g[:, i:i + 1], idx8[:, 0:1].bitcast(mybir.dt.int32)
                )

                # e = exp(2r - 2 rmax), s = accum
                bias_e = work.tile([PT, 1], f32, tag="bias_e")
                nc.vector.tensor_scalar_mul(bias_e, max8[:, 0:1], -2.0)
                e_sb = work.tile([128, K], bf16, tag="e")
                nc.scalar.activation(
                    e_sb, r_psum, AT.Exp,
                    bias=bias_e, scale=2.0, accum_out=s_all[:, i:i + 1],
                )

                # t = sum((-2 e) * r) = sum(e * d2) via stt accum
                ttr_scr = work.tile([128, K], bf16, tag="ttr")
                nc.vector.scalar_tensor_tensor(
                    out=ttr_scr,
                    in0=e_sb,
                    scalar=-2.0,
                    in1=r_psum,
                    op0=OP.mult,
                    op1=OP.mult,
                    accum_out=t_all[:, i:i + 1],
                )

                # gather + store centroids (single-column offsets only:
                # multi-column offset APs return wrong rows on HW)
                cent_sb = centp.tile([128, D], f32, tag="cent")
                nc.gpsimd.indirect_dma_start(
                    out=cent_sb,
                    out_offset=None,
                    in_=d_c[:, :],
                    in_offset=IndirectOffsetOnAxis(ap=idx_g[:, i:i + 1], axis=0),
                )
                nc.sync.dma_start(
                    d_cent.rearrange("(t p) d -> p t d", p=PT)[:, i, :],
                    cent_sb,
                )

            # ---- loss tail: loss_col[p] = sum_i t[p,i] / s[p,i] ----
            rec = small.tile([PT, NT], f32, tag="rec")
            nc.vector.reciprocal(rec, s_all)
            l_all = small.tile([PT, NT], f32, tag="l_all")
            nc.vector.tensor_mul(l_all, t_all, rec)
            l_scr = small.tile([PT, NT], f32, tag="l_scr")
            loss_col = small.tile([PT, 1], f32, tag="loss_col")
            nc.vector.tensor_scalar(
                l_scr, l_all, 1.0, 0.0, OP.mult, OP.add, accum_out=loss_col
            )
            nc.sync.dma_start(d_loss[:, :], loss_col)
            nc.sync.dma_start(d_idx[:, :], idx_g)

    nc.compile()
    return nc


def _get_nc():
    if "nc" not in _cached:
        _cached["nc"] = _build_bass()
    return _cached["nc"]


def kernel(input, cluster_rep):
    from concourse.bass_utils import run_bass_kernel_spmd

    x = np.ascontiguousarray(np.asarray(input, dtype=np.float32))
    c = np.ascontiguousarray(np.asarray(cluster_rep, dtype=np.float32))
    assert x.shape == (N, D) and c.shape == (K, D)

    nc = _get_nc()

    ct = np.ascontiguousarray(c.T)                      # [D, K] f32
    in_maps = []
    for core in range(NCORES):
        xs = x[core * NSHARD:(core + 1) * NSHARD]       # [NSHARD, D]
        xt = np.ascontiguousarray(xs.T)                 # [D, NSHARD]
        m = {"ct": ct, "c": c}
        if MM_MODE == "f16x3":
            xh = xt.astype(np.float16)
            xlo = (xt - xh.astype(np.float32)).astype(np.float16)
            chh = ct.astype(np.float16)
            cll = (ct - chh.astype(np.float32)).astype(np.float16)
            m.update({"xh": xh, "xl": xlo, "ch": chh, "cl": cll})
        else:
            m.update({"xr": xt, "cr": ct})
        in_maps.append(m)

    res = run_bass_kernel_spmd(nc, in_maps, core_ids=list(range(NCORES)))

    cent = np.concatenate([r["cent"] for r in res.results], axis=0)
    # idx dram layout [PT, NT]: global n = core*NSHARD + tile*PT + p
    idx = np.concatenate(
        [r["idx"].T.reshape(NSHARD) for r in res.results], axis=0
    ).astype(np.int32)
    loss = np.float32(
        sum(float(r["loss_col"].sum()) for r in res.results) / K
    )
    return cent, idx, loss


if __name__ == "__main__":
    # smoke test with random data
    rng = np.random.default_rng(0)
    x = rng.standard_normal((N, D)).astype(np.float32)
    c = rng.standard_normal((K, D)).astype(np.float32)
    cent, idx, loss = kernel(x, c)
    print(cent.shape, idx.shape, loss)


# revision 17
# speedup vs baseline: 1.2015x; 1.2015x over previous
"""TRN2 Bass kernel for nn_DkmCompGraph (vq_codebook).

reference:
    d2[n,k] = ||x_n||^2 + ||c_k||^2 - 2 x_n.c_k          (N=32768, K=1024, D=512)
    idx = argmin_k d2                                     -> (N,) int32
    centroids = cluster_rep[idx]                          -> (N, D) f32
    loss = sum(softmax(-d2, axis=1) * d2) / K             -> scalar f32

Strategy (8 cores, data-parallel over N; 4096 rows/core):
  - PE computes r = -d2/2 = x.c - ||x||^2/2 - ||c||^2/2 in PSUM via fp16
    hi/lo 3-term split matmuls (xh.ch + xh.cl + xl.ch, fp32-class accuracy)
    plus a K=3 bias matmul folding -||x||^2/2 (rank-1) and -||c||^2/2 rows.
  - argmin d2 = argmax r via DVE max (top-8) + max_index (first-occurrence
    ties, matching jnp.argmin).
  - softmax: e = exp(2r - 2 rmax) on ACT (scale=2, per-partition bias),
    accum_out gives s = sum_k e for free.
  - t = sum_k e * d2 = sum_k -2 e r via one fused tensor_tensor_reduce.
  - loss_row = t/s accumulated per partition; host sums 8x128 partials / K.
  - centroids: gpsimd indirect DMA row-gather from cluster_rep in DRAM.

Host side only reshards/transposes/splits inputs and concatenates outputs.
"""

import sys

if "/opt/trn_rl_repo" not in sys.path:
    sys.path.insert(0, "/opt/trn_rl_repo")

import numpy as np

N, D, K = 32768, 512, 1024
NCORES = 8
NSHARD = N // NCORES        # 4096
PT = 128                    # points per tile
NT = NSHARD // PT           # 32 tiles per core
DC = D // 128               # 4 contraction chunks
GATHER_GROUP = 4            # tiles per indirect-gather batch

# matmul mode: "f16x3" (safe) or "f32r" (fast, ~13-bit mantissa + host
# fixup of near-tie rows for exact argmin)
MM_MODE = "f32r"
# flag rows whose top-2 gap in r units is below this for exact host recheck
# (f32r dot error measured <= ~0.021 abs on this distribution; 2x margin)
GAP_THRESH = 0.1

_cached = {}


def _build_bass():
    from concourse import bacc
    import concourse.mybir as mybir
    import concourse.tile as tile
    from concourse.bass import IndirectOffsetOnAxis

    f16 = mybir.dt.float16
    f32 = mybir.dt.float32
    bf16 = mybir.dt.bfloat16
    f32r = mybir.dt.float32r
    AT = mybir.ActivationFunctionType
    OP = mybir.AluOpType

    nc = bacc.Bacc()

    if MM_MODE == "f16x3":
        d_xh = nc.dram_tensor("xh", [D, NSHARD], f16, kind="ExternalInput")
        d_xl = nc.dram_tensor("xl", [D, NSHARD], f16, kind="ExternalInput")
        d_ch = nc.dram_tensor("ch", [D, K], f16, kind="ExternalInput")
        d_cl = nc.dram_tensor("cl", [D, K], f16, kind="ExternalInput")
    else:
        d_xr = nc.dram_tensor("xr", [D, NSHARD], f32r, kind="ExternalInput")
        d_cr = nc.dram_tensor("cr", [D, K], f32r, kind="ExternalInput")
    d_ct = nc.dram_tensor("ct", [D, K], f32, kind="ExternalInput")   # cT fp32 (for c2)
    d_c = nc.dram_tensor("c", [K, D], f32, kind="ExternalInput")     # gather source

    d_cent = nc.dram_tensor("cent", [NSHARD, D], f32, kind="ExternalOutput")
    d_idx = nc.dram_tensor("idx", [PT, NT], mybir.dt.int32, kind="ExternalOutput")
    d_loss = nc.dram_tensor("loss_col", [PT, 1], f32, kind="ExternalOutput")
    if MM_MODE == "f32r":
        d_gap = nc.dram_tensor("gap", [PT, NT], f32, kind="ExternalOutput")

    with tile.TileContext(nc) as tc:
        with (
            tc.tile_pool(name="big", bufs=1) as big,        # persistent inputs
            tc.tile_pool(name="small", bufs=1) as small,    # persistent small
            tc.tile_pool(name="prep", bufs=2) as prep,      # transient prep
            tc.tile_pool(name="work", bufs=3) as work,      # per-tile work
            tc.tile_pool(name="cent", bufs=2) as centp,     # gather staging
            tc.tile_pool(name="ps", bufs=2, space="PSUM") as ps,
            tc.tile_pool(name="ps1", bufs=2, space="PSUM") as ps1,
            tc.tile_pool(name="psc", bufs=1, space="PSUM") as psc,
        ):
            # ---- persistent input loads (chunk-major [128, DC*n]) ----
            def load_T(dram, n, dt, tag):
                t = big.tile([128, DC * n], dt, tag=tag)
                nc.sync.dma_start(
                    t.rearrange("p (dc n) -> p dc n", dc=DC),
                    dram.rearrange("(dc p) n -> p dc n", p=128),
                )
                return t

            if MM_MODE == "f16x3":
                xh = load_T(d_xh, NSHARD, f16, "xh")
                xl = load_T(d_xl, NSHARD, f16, "xl")
                ch = load_T(d_ch, K, f16, "ch")
                cl = load_T(d_cl, K, f16, "cl")
            else:
                xr = load_T(d_xr, NSHARD, f32r, "xr")
                cr = load_T(d_cr, K, f32r, "cr")

            def chunk(t, dc, n, sl=slice(None)):
                return t[:, dc * n:(dc + 1) * n][:, sl]

            # ---- prep: c2row = sum_d c^2 (fp32 PE ones-matmul), bias rows ----
            ones_f32 = small.tile([128, 1], f32, tag="ones32")
            nc.vector.memset(ones_f32, 1.0)
            ones_f16 = small.tile([128, 1], f16, tag="ones16")
            nc.vector.memset(ones_f16, 1.0)

            c2_psum = psc.tile([1, K], f32)
            for dc in range(DC):
                ctc = prep.tile([128, K], f32, tag="ctc")
                nc.sync.dma_start(ctc, d_ct[dc * 128:(dc + 1) * 128, :])
                csq = prep.tile([128, K], f32, tag="csq")
                nc.scalar.activation(csq, ctc, AT.Square)
                for kh in range(2):
                    nc.tensor.matmul(
                        c2_psum[:, kh * 512:(kh + 1) * 512],
                        lhsT=ones_f32,
                        rhs=csq[:, kh * 512:(kh + 1) * 512],
                        start=(dc == 0),
                        stop=(dc == DC - 1),
                    )

            # bias_rhs [3, K] f16: rows = [ones, c2hi, c2lo] where c2hi+c2lo ~ -c2/2
            # (engine writes must start at partition 0/32/64/96, so compute the
            #  c2 rows in partition-0 tiles and DMA them into rows 1 and 2)
            bias_rhs = small.tile([3, K], f16, tag="brhs")
            nc.vector.memset(bias_rhs, 1.0)
            c2hi = small.tile([1, K], f16, tag="c2hi")
            nc.scalar.activation(c2hi, c2_psum, AT.Copy, scale=-0.5)
            c2lo_f32 = small.tile([1, K], f32, tag="c2lo")
            # c2lo = (-c2/2) - f16(-c2/2):  (scalar_tensor_tensor: (in0*-0.5) - hi)
            nc.vector.scalar_tensor_tensor(
                out=c2lo_f32,
                in0=c2_psum,
                scalar=-0.5,
                in1=c2hi,
                op0=OP.mult,
                op1=OP.subtract,
            )
            c2lo = small.tile([1, K], f16, tag="c2lo16")
            nc.vector.tensor_copy(c2lo, c2lo_f32)
            nc.sync.dma_start(bias_rhs[1:2, :], c2hi)
            nc.sync.dma_start(bias_rhs[2:3, :], c2lo)

            # bias_lhsT [3, PT] f16: rows = [negx2half (per tile), ones, ones]
            bias_lhsT = small.tile([3, PT], f16, tag="blhs")
            nc.vector.memset(bias_lhsT, 1.0)

            # in_max8 scaffold for max_index
            # (filled per-tile: col0 = rmax; cols 1..7 = -inf, never match)
            # built directly by nc.vector.max instead (true top-8).

            # per-row accumulators
            s_all = small.tile([PT, NT], f32, tag="s_all")
            t_all = small.tile([PT, NT], f32, tag="t_all")
            idx_g = small.tile([PT, NT], mybir.dt.int32, tag="idx_g")
            if MM_MODE == "f32r":
                gap_all = small.tile([PT, NT], f32, tag="gap_all")

            # ---- main loop ----
            for i in range(NT):
                psl = slice(i * PT, (i + 1) * PT)

                # x2 = sum_d xh^2 (f16): one ACT square + 4 accumulating ones-mms
                if MM_MODE == "f16x3":
                    sq_src = xh
                    sq_dt = f16
                else:
                    sq_src = xr
                    sq_dt = f16
                sq = work.tile([128, DC * PT], sq_dt, tag="sq")
                nc.scalar.activation(
                    sq.rearrange("p (dc n) -> p dc n", dc=DC),
                    sq_src.rearrange("p (dc n) -> p dc n", dc=DC)[:, :, psl],
                    AT.Square,
                )
                x2_psum = ps1.tile([1, PT], f32)
                for dc in range(DC):
                    nc.tensor.matmul(
                        x2_psum,
                        lhsT=ones_f16,
                        rhs=chunk(sq, dc, PT),
                        start=(dc == 0),
                        stop=(dc == DC - 1),
                    )
                # bias_lhsT row 0 = -x2/2 (f16)
                nc.scalar.activation(bias_lhsT[0:1, :], x2_psum, AT.Copy, scale=-0.5)

                # main matmuls -> r = -d2/2 in PSUM [128, K] f32
                r_psum = ps.tile([128, K], f32)
                for kh in range(2):
                    ksl = slice(kh * 512, (kh + 1) * 512)
                    if MM_MODE == "f16x3":
                        pairs = [
                            (a, b)
                            for dc in range(DC)
                            for (a, b) in (
                                (chunk(xh, dc, NSHARD, psl), chunk(ch, dc, K, ksl)),
                                (chunk(xh, dc, NSHARD, psl), chunk(cl, dc, K, ksl)),
                                (chunk(xl, dc, NSHARD, psl), chunk(ch, dc, K, ksl)),
                            )
                        ]
                    else:
                        pairs = [
                            (chunk(xr, dc, NSHARD, psl), chunk(cr, dc, K, ksl))
                            for dc in range(DC)
                        ]
                    for j, (a, b) in enumerate(pairs):
                        nc.tensor.matmul(
                            r_psum[:, ksl], lhsT=a, rhs=b,
                            start=(j == 0), stop=False,
                        )
                    nc.tensor.matmul(
                        r_psum[:, ksl],
                        lhsT=bias_lhsT,
                        rhs=bias_rhs[:, ksl],
                        start=False,
                        stop=True,
                    )

                # top-8 + argmax (first occurrence on ties = jnp.argmin order)
                max8 = work.tile([PT, 8], f32, tag="max8")
                nc.vector.max(out=max8, in_=r_psum)
                idx8 = work.tile([PT, 8], mybir.dt.uint32, tag="idx8")
                nc.vector.max_index(out=idx8, in_max=max8, in_values=r_psum)
                nc.vector.tensor_copy(
                    idx_g[:, i:i + 1], idx8[:, 0:1].bitcast(mybir.dt.int32)
                )
                if MM_MODE == "f32r":
                    nc.vector.tensor_sub(
                        gap_all[:, i:i + 1], max8[:, 0:1], max8[:, 1:2]
                    )

                # e = exp(2r - 2 rmax), s = accum
                bias_e = work.tile([PT, 1], f32, tag="bias_e")
                nc.vector.tensor_scalar_mul(bias_e, max8[:, 0:1], -2.0)
                e_sb = work.tile([128, K], bf16, tag="e")
                nc.scalar.activation(
                    e_sb, r_psum, AT.Exp,
                    bias=bias_e, scale=2.0, accum_out=s_all[:, i:i + 1],
                )

                # t = sum((-2 e) * r) = sum(e * d2) via stt accum
                ttr_scr = work.tile([128, K], bf16, tag="ttr")
                nc.vector.scalar_tensor_tensor(
                    out=ttr_scr,
                    in0=e_sb,
                    scalar=-2.0,
                    in1=r_psum,
                    op0=OP.mult,
                    op1=OP.mult,
                    accum_out=t_all[:, i:i + 1],
                )

                # gather + store centroids (single-column offsets only:
                # multi-column offset APs return wrong rows on HW)
                cent_sb = centp.tile([128, D], f32, tag="cent")
                nc.gpsimd.indirect_dma_start(
                    out=cent_sb,
                    out_offset=None,
                    in_=d_c[:, :],
                    in_offset=IndirectOffsetOnAxis(ap=idx_g[:, i:i + 1], axis=0),
                )
                nc.sync.dma_start(
                    d_cent.rearrange("(t p) d -> p t d", p=PT)[:, i, :],
                    cent_sb,
                )

            # ---- loss tail: loss_col[p] = sum_i t[p,i] / s[p,i] ----
            rec = small.tile([PT, NT], f32, tag="rec")
            nc.vector.reciprocal(rec, s_all)
            l_all = small.tile([PT, NT], f32, tag="l_all")
            nc.vector.tensor_mul(l_all, t_all, rec)
            l_scr = small.tile([PT, NT], f32, tag="l_scr")
            loss_col = small.tile([PT, 1], f32, tag="loss_col")
            nc.vector.tensor_scalar(
                l_scr, l_all, 1.0, 0.0, OP.mult, OP.add, accum_out=loss_col
            )
            nc.sync.dma_start(d_loss[:, :], loss_col)
            nc.sync.dma_start(d_idx[:, :], idx_g)
            if MM_MODE == "f32r":
                nc.sync.dma_start(d_gap[:, :], gap_all)

    nc.compile()
    return nc


def _get_nc():
    if "nc" not in _cached:
        _cached["nc"] = _build_bass()
    return _cached["nc"]


def kernel(input, cluster_rep):
    from concourse.bass_utils import run_bass_kernel_spmd

    x = np.ascontiguousarray(np.asarray(input, dtype=np.float32))
    c = np.ascontiguousarray(np.asarray(cluster_rep, dtype=np.float32))
    assert x.shape == (N, D) and c.shape == (K, D)

    nc = _get_nc()

    ct = np.ascontiguousarray(c.T)                      # [D, K] f32
    in_maps = []
    for core in range(NCORES):
        xs = x[core * NSHARD:(core + 1) * NSHARD]       # [NSHARD, D]
        xt = np.ascontiguousarray(xs.T)                 # [D, NSHARD]
        m = {"ct": ct, "c": c}
        if MM_MODE == "f16x3":
            xh = xt.astype(np.float16)
            xlo = (xt - xh.astype(np.float32)).astype(np.float16)
            chh = ct.astype(np.float16)
            cll = (ct - chh.astype(np.float32)).astype(np.float16)
            m.update({"xh": xh, "xl": xlo, "ch": chh, "cl": cll})
        else:
            m.update({"xr": xt, "cr": ct})
        in_maps.append(m)

    res = run_bass_kernel_spmd(nc, in_maps, core_ids=list(range(NCORES)))

    cent = np.concatenate([r["cent"] for r in res.results], axis=0)
    # idx dram layout [PT, NT]: global n = core*NSHARD + tile*PT + p
    idx = np.concatenate(
        [r["idx"].T.reshape(NSHARD) for r in res.results], axis=0
    ).astype(np.int32)
    loss = np.float32(
        sum(float(r["loss_col"].sum()) for r in res.results) / K
    )

    if MM_MODE == "f32r":
        # exact recheck of rows whose top-2 distance gap is within the f32r
        # matmul error bound; fixes potential argmin flips
        gap = np.concatenate(
            [r["gap"].T.reshape(NSHARD) for r in res.results], axis=0
        )
        rows = np.nonzero(gap < GAP_THRESH)[0]
        if rows.size:
            xr64 = x[rows].astype(np.float64)
            c64 = c.astype(np.float64)
            d2r = (
                (xr64 * xr64).sum(1)[:, None]
                + (c64 * c64).sum(1)[None, :]
                - 2.0 * (xr64 @ c64.T)
            )
            idx_fix = np.argmin(d2r, axis=1).astype(np.int32)
            changed = idx_fix != idx[rows]
            if __name__ != "__main__":
                import os
                if os.environ.get("VQ_DEBUG"):
                    print(f"[fixup] flagged {rows.size} rows, "
                          f"changed {int(changed.sum())}")
            if changed.any():
                upd = rows[changed]
                idx[upd] = idx_fix[changed]
                cent[upd] = c[idx[upd]]

    return cent, idx, loss


if __name__ == "__main__":
    # smoke test with random data
    rng = np.random.default_rng(0)
    x = rng.standard_normal((N, D)).astype(np.float32)
    c = rng.standard_normal((K, D)).astype(np.float32)
    cent, idx, loss = kernel(x, c)
    print(cent.shape, idx.shape, loss)
